# revision 1
# baseline (speedup 1.0000x reference)
import numpy as np
import jax
import jax.numpy as jnp

# nn_HWTConv2D: B=16, C=64, H=W=256, P=2 pods. Data-parallel over batch on 8 cores.
B, C, H, W, P = 16, 64, 256, 256, 2
NCORES = 8
NORM = float(1.0 / np.sqrt(2.0))


def _haar_matrix(n):
    # Orthonormal multilevel 1D Haar matrix: haar1d_fwd(x) == Hm @ x.
    m = int(np.log2(n))
    Hm = np.eye(n, dtype=np.float64)
    length = n
    for _ in range(m):
        L = np.eye(n, dtype=np.float64)
        half = length // 2
        blk = np.zeros((length, length), dtype=np.float64)
        for i in range(half):
            blk[i, 2 * i] = NORM
            blk[i, 2 * i + 1] = NORM
            blk[half + i, 2 * i] = NORM
            blk[half + i, 2 * i + 1] = -NORM
        L[:length, :length] = blk
        Hm = L @ Hm
        length //= 2
    return Hm.astype(np.float32)


_HM = _haar_matrix(H)  # (256, 256), orthonormal: inverse = HM.T


def _shard_fn(x, v, conv_w, tau, hm, hmT):
    # x: (B/8, C, H, W). F = hm @ X @ hmT applied per (b, c) plane.
    hp = jax.lax.Precision.HIGHEST
    f1 = jnp.matmul(jnp.matmul(hm, x, precision=hp), hmT, precision=hp)
    acc = f1
    for i in range(P):
        f3 = (f1 * v[i]).reshape(x.shape[0], C, H * W)
        f4 = jnp.matmul(conv_w[i], f3, precision=hp).reshape(x.shape)
        f5 = f4 - jnp.clip(f4, -tau[i], tau[i])
        acc = acc + f5
    # residual folded in wavelet domain (acc started from f1): y = hmT(acc)hm
    return jnp.matmul(jnp.matmul(hmT, acc, precision=hp), hm, precision=hp)


_jitted = jax.jit(_shard_fn)


def kernel(x, v, conv_w, tau):
    devs = jax.devices()[:NCORES]
    xs = x.reshape(NCORES, B // NCORES, C, H, W)
    hmT = np.ascontiguousarray(_HM.T)
    outs = []
    for d in range(NCORES):
        args = [jax.device_put(a, devs[d]) for a in (xs[d], v, conv_w, tau, _HM, hmT)]
        outs.append(_jitted(*args))
    y = np.concatenate([np.asarray(o) for o in outs], axis=0)
    return y.reshape(B, C, H, W).astype(np.float32)



# revision 18
# speedup vs baseline: 1.5455x; 1.5455x over previous
"""nn_HWTConv2D Trainium2 kernel.

y = x + iHaar2d( sum_p SoftThresh( conv1x1_p( Haar2d(x) * v_p ), tau_p ) )

Haar2d(x) = HM @ X @ HM^T per (b, c) plane with the orthonormal multilevel
Haar matrix HM (256x256), so the whole pipeline is matmuls + elementwise.

Distribution: data-parallel over batch, B=16 -> 2 batches per core on 8
NeuronCores.  The device computes corr = y - x in bf16; the host adds the
fp32 residual x, which keeps the wire traffic at bf16 both ways (the axon
tunnel at ~60 MB/s is the real bottleneck, not the device).

Per-core bass program (all matmuls bf16, PSUM fp32):
  S1/S2  forward transform, data-stationary matmuls (lhsT = data tile,
         rhs = HM^T tiles): each stage contracts the partition dim and
         transposes the plane, so two stages return to [h'|w'] layout.
  conv   channels must sit on partitions: small DMA gathers build
         [c=64 | pix] tiles from F1, one stationary W^T per pod, and the
         [o | pix] results are DMA-scattered back to [h' | (o, w')].
  thresh f5 = t - clip(t, +-tau) with t = q * v, computed as
         m = min(t, tau); m2 = min(-m, tau) = -clip; f5 = t + m2,
         using v/tau slices broadcast along the free (o) dim.
  I_h/I_w inverse transform, same data-stationary structure, writes
         corr planes [h | w] contiguously to HBM.
"""

import threading

import numpy as np
import ml_dtypes

B, C, H, W, P = 16, 64, 256, 256, 2
NCORES = 8
BPC = B // NCORES  # batches per core
BF16 = ml_dtypes.bfloat16
NORM = float(1.0 / np.sqrt(2.0))

_lock = threading.Lock()
_state: dict = {}


def _haar_matrix(n):
    # Orthonormal multilevel 1D Haar matrix: haar1d_fwd(x) == HM @ x.
    m = int(np.log2(n))
    hm = np.eye(n, dtype=np.float64)
    length = n
    for _ in range(m):
        lvl = np.eye(n, dtype=np.float64)
        half = length // 2
        blk = np.zeros((length, length), dtype=np.float64)
        for i in range(half):
            blk[i, 2 * i] = NORM
            blk[i, 2 * i + 1] = NORM
            blk[half + i, 2 * i] = NORM
            blk[half + i, 2 * i + 1] = -NORM
        lvl[:length, :length] = blk
        hm = lvl @ hm
        length //= 2
    return hm.astype(np.float32)


def _build_nc():
    import concourse.bacc as bacc
    import concourse.tile as tile
    from concourse import mybir
    from contextlib import ExitStack

    dt = mybir.dt
    alu = mybir.AluOpType
    nc = bacc.Bacc("TRN2", target_bir_lowering=False, debug=False)

    xs = nc.dram_tensor("xs", [BPC, C, 128, 2, 256], dt.bfloat16, kind="ExternalInput")
    hmt_s = nc.dram_tensor("hmt_s", [128, 2, 256], dt.bfloat16, kind="ExternalInput")
    hm_s = nc.dram_tensor("hm_s", [128, 2, 256], dt.bfloat16, kind="ExternalInput")
    wdup = nc.dram_tensor("wdup", [64, P, 64], dt.bfloat16, kind="ExternalInput")
    v_s = nc.dram_tensor("v_s", [128, P, 2, 256], dt.bfloat16, kind="ExternalInput")
    tau_s = nc.dram_tensor("tau_s", [128, P, 2, 256], dt.bfloat16, kind="ExternalInput")
    corr = nc.dram_tensor("corr", [BPC, C, 2, 128, 256], dt.bfloat16, kind="ExternalOutput")

    with ExitStack() as ctx:
        tc = ctx.enter_context(tile.TileContext(nc))
        pc = ctx.enter_context(tc.tile_pool(name="consts", bufs=1))
        pw = ctx.enter_context(tc.tile_pool(name="work", bufs=1))
        pps = ctx.enter_context(tc.tile_pool(name="ps", bufs=4, space="PSUM"))
        pps3 = ctx.enter_context(tc.tile_pool(name="ps3", bufs=4, space="PSUM"))

        hmt_sb = pc.tile_from(hmt_s[:])
        hm_sb = pc.tile_from(hm_s[:])
        wdup_sb = pc.tile_from(wdup[:])
        v_sb = pc.tile_from(v_s[:])
        tau_sb = pc.tile_from(tau_s[:])

        # Every working tile is allocated exactly once (static address) and
        # reused round-robin: only same-tensor dependency tracking is needed.
        f1 = pw.tile([128, 2, C, 256], dt.bfloat16, tag="f1")
        q = pw.tile([128, 2, C, 256], dt.bfloat16, tag="q")
        q2 = pw.tile([128, C, 256], dt.bfloat16, tag="q2")
        xcs = [pw.tile([128, 2, 256], dt.bfloat16, tag=f"xc{i}", name=f"xc{i}") for i in range(3)]
        r1s_ = [pw.tile([128, 256], dt.bfloat16, tag=f"r1{i}", name=f"r1{i}") for i in range(4)]
        ftcs = [pw.tile([64, 8 * 256], dt.bfloat16, tag=f"ftc{i}", name=f"ftc{i}") for i in range(2)]
        sgs = [pw.tile([64, 8 * 256], dt.bfloat16, tag=f"sg{i}", name=f"sg{i}") for i in range(2)]
        tts = [pw.tile([128, 8, 256], dt.bfloat16, tag=f"tt{i}", name=f"tt{i}") for i in range(2)]
        mms = [pw.tile([128, 8, 256], dt.bfloat16, tag=f"mm{i}", name=f"mm{i}") for i in range(2)]
        t2s = [pw.tile([128, 2, 256], dt.bfloat16, tag=f"t2{i}", name=f"t2{i}") for i in range(2)]
        outps = [pw.tile([128, 256], dt.bfloat16, tag=f"outp{i}", name=f"outp{i}") for i in range(4)]

        for b in range(BPC):
            # ---- forward transform: per-plane, fused S1+S2 ----
            for c in range(C):
                xc = xcs[c % 3]
                nc.gpsimd.dma_start(xc[:], xs[b, c])
                r1s = []
                for wh in range(2):
                    ps1 = pps.tile([128, 256], dt.float32, tag="ps")
                    nc.tensor.matmul(ps1[:], xc[:, 0, wh * 128:(wh + 1) * 128],
                                     hmt_sb[:, 0, :], start=True, stop=False)
                    nc.tensor.matmul(ps1[:], xc[:, 1, wh * 128:(wh + 1) * 128],
                                     hmt_sb[:, 1, :], start=False, stop=True)
                    r1 = r1s_[(2 * c + wh) % 4]
                    nc.any.tensor_copy(r1[:], ps1[:])
                    r1s.append(r1)
                for hph in range(2):
                    ps2 = pps.tile([128, 256], dt.float32, tag="ps")
                    nc.tensor.matmul(ps2[:], r1s[0][:, hph * 128:(hph + 1) * 128],
                                     hmt_sb[:, 0, :], start=True, stop=False)
                    nc.tensor.matmul(ps2[:], r1s[1][:, hph * 128:(hph + 1) * 128],
                                     hmt_sb[:, 1, :], start=False, stop=True)
                    nc.any.tensor_copy(f1[:, hph, c, :], ps2[:])

            # ---- conv (channel mix) + soft-threshold ----
            for hph in range(2):
                for chk in range(16):
                    ftc = ftcs[chk % 2]
                    ftv = ftc[:].rearrange("c (hl w) -> c hl w", hl=8)
                    for hl in range(8):
                        row = chk * 8 + hl
                        nc.gpsimd.dma_start(ftv[:, hl, :], f1[row:row + 1, hph, :, :])
                    for pod in range(P):
                        sg = sgs[pod]
                        for q4 in range(4):
                            ps3 = pps3.tile([64, 512], dt.float32, tag="ps3")
                            nc.tensor.matmul(ps3[:], wdup_sb[:, pod, :],
                                             ftc[:, q4 * 512:(q4 + 1) * 512],
                                             start=True, stop=True)
                            nc.any.tensor_copy(sg[:, q4 * 512:(q4 + 1) * 512], ps3[:])
                        dst = q if pod == 0 else q2
                        sgv = sg[:].rearrange("o (hl w) -> o hl w", hl=8)
                        for hl in range(8):
                            row = chk * 8 + hl
                            drow = (dst[row:row + 1, hph, :, :] if pod == 0
                                    else dst[row:row + 1, :, :])
                            nc.gpsimd.dma_start(drow, sgv[:, hl, :])
                for ch2 in range(8):
                    osl = slice(ch2 * 8, (ch2 + 1) * 8)
                    qs = q[:, hph, osl, :]
                    q2s = q2[:, osl, :]
                    t = tts[ch2 % 2]
                    m = mms[ch2 % 2]
                    vb0 = v_sb[:, 0, hph, :].unsqueeze(1).broadcast_to([128, 8, 256])
                    tb0 = tau_sb[:, 0, hph, :].unsqueeze(1).broadcast_to([128, 8, 256])
                    vb1 = v_sb[:, 1, hph, :].unsqueeze(1).broadcast_to([128, 8, 256])
                    tb1 = tau_sb[:, 1, hph, :].unsqueeze(1).broadcast_to([128, 8, 256])
                    # pod0, in place: q <- t + min(-min(t,tau), tau) = t - clip(t)
                    nc.vector.tensor_tensor(t[:], qs, vb0, alu.mult)
                    nc.vector.tensor_tensor(m[:], t[:], tb0, alu.min)
                    nc.vector.scalar_tensor_tensor(m[:], m[:], -1.0, tb0, alu.mult, alu.min)
                    nc.vector.tensor_tensor(qs, t[:], m[:], alu.add)
                    # pod1, accumulate into q
                    nc.vector.tensor_tensor(t[:], q2s, vb1, alu.mult)
                    nc.vector.tensor_tensor(m[:], t[:], tb1, alu.min)
                    nc.vector.scalar_tensor_tensor(m[:], m[:], -1.0, tb1, alu.mult, alu.min)
                    nc.vector.tensor_tensor(t[:], t[:], m[:], alu.add)
                    nc.vector.tensor_tensor(qs, qs, t[:], alu.add)

            # ---- inverse transform, per plane ----
            for o in range(C):
                t2 = t2s[o % 2]
                for wph in range(2):
                    ps = pps.tile([128, 256], dt.float32, tag="ps")
                    nc.tensor.matmul(ps[:], q[:, 0, o, wph * 128:(wph + 1) * 128],
                                     hm_sb[:, 0, :], start=True, stop=False)
                    nc.tensor.matmul(ps[:], q[:, 1, o, wph * 128:(wph + 1) * 128],
                                     hm_sb[:, 1, :], start=False, stop=True)
                    nc.any.tensor_copy(t2[:, wph, :], ps[:])
                for hh in range(2):
                    ps = pps.tile([128, 256], dt.float32, tag="ps")
                    nc.tensor.matmul(ps[:], t2[:, 0, hh * 128:(hh + 1) * 128],
                                     hm_sb[:, 0, :], start=True, stop=False)
                    nc.tensor.matmul(ps[:], t2[:, 1, hh * 128:(hh + 1) * 128],
                                     hm_sb[:, 1, :], start=False, stop=True)
                    outp = outps[(2 * o + hh) % 4]
                    nc.any.tensor_copy(outp[:], ps[:])
                    nc.gpsimd.dma_start(corr[b, o, hh], outp[:])

    nc.compile()
    nc.finalize()
    return nc


def _prep_consts(v, conv_w, tau):
    hm = _haar_matrix(H)
    hmt = np.ascontiguousarray(hm.T)
    hmt_s = np.ascontiguousarray(hmt.reshape(2, 128, 256).transpose(1, 0, 2)).astype(BF16)
    hm_s = np.ascontiguousarray(hm.reshape(2, 128, 256).transpose(1, 0, 2)).astype(BF16)
    wdup = np.ascontiguousarray(conv_w.transpose(2, 0, 1)).astype(BF16)  # [c, pod, o]
    v_s = np.ascontiguousarray(
        v.reshape(P, 2, 128, 256).transpose(2, 0, 1, 3)).astype(BF16)
    tau_s = np.ascontiguousarray(
        tau.reshape(P, 2, 128, 256).transpose(2, 0, 1, 3)).astype(BF16)
    return {"hmt_s": hmt_s, "hm_s": hm_s, "wdup": wdup, "v_s": v_s, "tau_s": tau_s}


def _prep_x(x):
    # [B, C, H, W] fp32 -> [B, C, 128, 2, 256] bf16 with h = hh*128 + p
    xr = x.reshape(B, C, 2, 128, 256).transpose(0, 1, 3, 2, 4)
    return np.ascontiguousarray(xr).astype(BF16)


def _build_fast(nc):
    """Cached jit(shard_map) executing the bass NEFF on 8 cores.

    Same mechanism as run_bass_kernel_spmd's axon path (bass2jax custom
    call), but built once (stable jit cache) and with the NEFF output
    buffers created on-device via jnp.zeros instead of shipping
    134MB of zeros over the axon wire on every call.
    """
    import jax
    import jax.numpy as jnp
    from jax.sharding import Mesh, PartitionSpec
    try:
        from jax.experimental.shard_map import shard_map
    except ImportError:
        from jax.shard_map import shard_map  # newer jax
    from concourse import bass2jax, mybir

    bass2jax.install_neuronx_cc_hook()

    pname = nc.partition_id_tensor.name if nc.partition_id_tensor else None
    in_names, out_names, out_avals = [], [], []
    for alloc in nc.m.functions[0].allocations:
        if not isinstance(alloc, mybir.MemoryLocationSet):
            continue
        name = alloc.memorylocations[0].name
        if alloc.kind == "ExternalInput":
            if name != pname:
                in_names.append(name)
        elif alloc.kind == "ExternalOutput":
            out_names.append(name)
            out_avals.append(jax.core.ShapedArray(
                tuple(alloc.tensor_shape), mybir.dt.np(alloc.dtype)))

    bind_names = list(in_names) + list(out_names)
    if pname is not None:
        bind_names.append(pname)

    def _body(*args):
        # args = per-core inputs followed by per-core zero output buffers
        operands = list(args)
        if pname is not None:
            operands.append(bass2jax.partition_id_tensor())
        outs = bass2jax._bass_exec_p.bind(
            *operands,
            out_avals=tuple(out_avals),
            in_names=tuple(bind_names),
            out_names=tuple(out_names),
            lowering_input_output_aliases=(),
            sim_require_finite=True,
            sim_require_nnan=True,
            nc=nc,
        )
        return tuple(outs)

    devices = jax.devices()[:NCORES]
    mesh = Mesh(np.asarray(devices), ("core",))
    n_args = len(in_names) + len(out_avals)
    fast = jax.jit(shard_map(
        _body, mesh=mesh,
        in_specs=(PartitionSpec("core"),) * n_args,
        out_specs=(PartitionSpec("core"),) * len(out_names),
        check_rep=False))
    # device-resident zero output buffers, created once: passing them as
    # arguments keeps the custom-call operands parameter-only (the
    # neuronx_cc_hook rejects constants) without re-shipping 134MB of
    # zeros over the axon wire on every call.
    from jax.sharding import NamedSharding
    sh = NamedSharding(mesh, PartitionSpec("core"))
    zeros_dev = [
        jax.device_put(
            np.zeros((NCORES * a.shape[0], *a.shape[1:]), a.dtype), sh)
        for a in out_avals
    ]
    for z in zeros_dev:
        z.block_until_ready()
    return fast, in_names, out_names, zeros_dev


def _concat_inputs(in_names, per_core):
    return [np.concatenate([per_core[c][n] for c in range(NCORES)], axis=0)
            for n in in_names]


def kernel(x, v, conv_w, tau):
    x = np.asarray(x, dtype=np.float32)
    consts = _prep_consts(np.asarray(v, np.float32),
                          np.asarray(conv_w, np.float32),
                          np.asarray(tau, np.float32))
    xh = _prep_x(x)  # [B, C, 128, 2, 256] bf16
    xsh = xh.reshape(NCORES, BPC, C, 128, 2, 256)
    per_core = [dict(consts, xs=np.ascontiguousarray(xsh[i])) for i in range(NCORES)]

    with _lock:
        if "fast" not in _state:
            from concourse.bass_utils import run_bass_kernel_spmd
            nc = _build_nc()
            res = run_bass_kernel_spmd(nc, per_core, list(range(NCORES)))
            corr = np.stack([res.results[i]["corr"] for i in range(NCORES)])
            _state["fast"] = _build_fast(nc)
        else:
            fast, in_names, out_names, zeros_dev = _state["fast"]
            outs = fast(*_concat_inputs(in_names, per_core), *zeros_dev)
            corr = np.asarray(outs[out_names.index("corr")]).reshape(
                NCORES, BPC, C, 2, 128, 256)

    # corr: [cores, BPC, C, 2, 128, 256] bf16, (hh, p) -> h
    corr = corr.reshape(B, C, H, W).astype(np.float32)
    return (x + corr).astype(np.float32)


# revision 21
# speedup vs baseline: 2.5328x; 1.6388x over previous
"""nn_HWTConv2D Trainium2 kernel.

y = x + iHaar2d( sum_p SoftThresh( conv1x1_p( Haar2d(x) * v_p ), tau_p ) )

Haar2d(x) = HM @ X @ HM^T per (b, c) plane with the orthonormal multilevel
Haar matrix HM (256x256), so the whole pipeline is matmuls + elementwise.

Distribution: data-parallel over batch, B=16 -> 2 batches per core on 8
NeuronCores.  The device computes corr = y - x in bf16; the host adds the
fp32 residual x, which keeps the wire traffic at bf16 both ways (the axon
tunnel at ~60 MB/s is the real bottleneck, not the device).

Per-core bass program (all matmuls bf16, PSUM fp32):
  S1/S2  forward transform, data-stationary matmuls (lhsT = data tile,
         rhs = HM^T tiles): each stage contracts the partition dim and
         transposes the plane, so two stages return to [h'|w'] layout.
  conv   channels must sit on partitions: small DMA gathers build
         [c=64 | pix] tiles from F1, one stationary W^T per pod, and the
         [o | pix] results are DMA-scattered back to [h' | (o, w')].
  thresh f5 = t - clip(t, +-tau) with t = q * v, computed as
         m = min(t, tau); m2 = min(-m, tau) = -clip; f5 = t + m2,
         using v/tau slices broadcast along the free (o) dim.
  I_h/I_w inverse transform, same data-stationary structure, writes
         corr planes [h | w] contiguously to HBM.
"""

import threading

import numpy as np
import ml_dtypes

B, C, H, W, P = 16, 64, 256, 256, 2
NCORES = 8
BPC = B // NCORES  # batches per core
BF16 = ml_dtypes.bfloat16
NORM = float(1.0 / np.sqrt(2.0))

_lock = threading.Lock()
_state: dict = {}


def _haar_matrix(n):
    # Orthonormal multilevel 1D Haar matrix: haar1d_fwd(x) == HM @ x.
    m = int(np.log2(n))
    hm = np.eye(n, dtype=np.float64)
    length = n
    for _ in range(m):
        lvl = np.eye(n, dtype=np.float64)
        half = length // 2
        blk = np.zeros((length, length), dtype=np.float64)
        for i in range(half):
            blk[i, 2 * i] = NORM
            blk[i, 2 * i + 1] = NORM
            blk[half + i, 2 * i] = NORM
            blk[half + i, 2 * i + 1] = -NORM
        lvl[:length, :length] = blk
        hm = lvl @ hm
        length //= 2
    return hm.astype(np.float32)


def _build_nc():
    import concourse.bacc as bacc
    import concourse.tile as tile
    from concourse import mybir
    from contextlib import ExitStack

    dt = mybir.dt
    alu = mybir.AluOpType
    nc = bacc.Bacc("TRN2", target_bir_lowering=False, debug=False)

    xs = nc.dram_tensor("xs", [BPC, C, 2, 128, 256], dt.bfloat16, kind="ExternalInput")
    hmt_s = nc.dram_tensor("hmt_s", [128, 2, 256], dt.bfloat16, kind="ExternalInput")
    hm_s = nc.dram_tensor("hm_s", [128, 2, 256], dt.bfloat16, kind="ExternalInput")
    wdup = nc.dram_tensor("wdup", [64, P, 64], dt.bfloat16, kind="ExternalInput")
    v_s = nc.dram_tensor("v_s", [128, P, 2, 256], dt.bfloat16, kind="ExternalInput")
    tau_s = nc.dram_tensor("tau_s", [128, P, 2, 256], dt.bfloat16, kind="ExternalInput")
    corr = nc.dram_tensor("corr", [BPC, C, 2, 128, 256], dt.float8e4, kind="ExternalOutput")

    with ExitStack() as ctx:
        tc = ctx.enter_context(tile.TileContext(nc))
        pc = ctx.enter_context(tc.tile_pool(name="consts", bufs=1))
        pw = ctx.enter_context(tc.tile_pool(name="work", bufs=1))
        pps = ctx.enter_context(tc.tile_pool(name="ps", bufs=4, space="PSUM"))
        pps3 = ctx.enter_context(tc.tile_pool(name="ps3", bufs=4, space="PSUM"))

        hmt_sb = pc.tile_from(hmt_s[:])
        hm_sb = pc.tile_from(hm_s[:])
        wdup_sb = pc.tile_from(wdup[:])
        v_sb = pc.tile_from(v_s[:])
        tau_sb = pc.tile_from(tau_s[:])

        # Every working tile is allocated exactly once (static address) and
        # reused round-robin: only same-tensor dependency tracking is needed.
        f1 = pw.tile([128, 2, C, 256], dt.bfloat16, tag="f1")
        q = pw.tile([128, 2, C, 256], dt.bfloat16, tag="q")
        q2 = pw.tile([128, C, 256], dt.bfloat16, tag="q2")
        xcs = [pw.tile([128, 2, 256], dt.bfloat16, tag=f"xc{i}", name=f"xc{i}") for i in range(3)]
        r1s_ = [pw.tile([128, 256], dt.bfloat16, tag=f"r1{i}", name=f"r1{i}") for i in range(4)]
        ftcs = [pw.tile([64, 8 * 256], dt.bfloat16, tag=f"ftc{i}", name=f"ftc{i}") for i in range(2)]
        sgs = [pw.tile([64, 8 * 256], dt.bfloat16, tag=f"sg{i}", name=f"sg{i}") for i in range(2)]
        tts = [pw.tile([128, 8, 256], dt.bfloat16, tag=f"tt{i}", name=f"tt{i}") for i in range(2)]
        mms = [pw.tile([128, 8, 256], dt.bfloat16, tag=f"mm{i}", name=f"mm{i}") for i in range(2)]
        t2s = [pw.tile([128, 2, 256], dt.bfloat16, tag=f"t2{i}", name=f"t2{i}") for i in range(2)]
        outps = [pw.tile([128, 256], dt.float8e4, tag=f"outp{i}", name=f"outp{i}") for i in range(4)]

        for b in range(BPC):
            # ---- forward transform: per-plane, fused S1+S2 ----
            for c in range(C):
                xc = xcs[c % 3]
                for hh in range(2):
                    nc.gpsimd.dma_start(xc[:, hh, :], xs[b, c, hh])
                r1s = []
                for wh in range(2):
                    ps1 = pps.tile([128, 256], dt.float32, tag="ps")
                    nc.tensor.matmul(ps1[:], xc[:, 0, wh * 128:(wh + 1) * 128],
                                     hmt_sb[:, 0, :], start=True, stop=False)
                    nc.tensor.matmul(ps1[:], xc[:, 1, wh * 128:(wh + 1) * 128],
                                     hmt_sb[:, 1, :], start=False, stop=True)
                    r1 = r1s_[(2 * c + wh) % 4]
                    nc.any.tensor_copy(r1[:], ps1[:])
                    r1s.append(r1)
                for hph in range(2):
                    ps2 = pps.tile([128, 256], dt.float32, tag="ps")
                    nc.tensor.matmul(ps2[:], r1s[0][:, hph * 128:(hph + 1) * 128],
                                     hmt_sb[:, 0, :], start=True, stop=False)
                    nc.tensor.matmul(ps2[:], r1s[1][:, hph * 128:(hph + 1) * 128],
                                     hmt_sb[:, 1, :], start=False, stop=True)
                    nc.any.tensor_copy(f1[:, hph, c, :], ps2[:])

            # ---- conv (channel mix) + soft-threshold ----
            for hph in range(2):
                for chk in range(16):
                    ftc = ftcs[chk % 2]
                    ftv = ftc[:].rearrange("c (hl w) -> c hl w", hl=8)
                    for hl in range(8):
                        row = chk * 8 + hl
                        nc.gpsimd.dma_start(ftv[:, hl, :], f1[row:row + 1, hph, :, :])
                    for pod in range(P):
                        sg = sgs[pod]
                        for q4 in range(4):
                            ps3 = pps3.tile([64, 512], dt.float32, tag="ps3")
                            nc.tensor.matmul(ps3[:], wdup_sb[:, pod, :],
                                             ftc[:, q4 * 512:(q4 + 1) * 512],
                                             start=True, stop=True)
                            nc.any.tensor_copy(sg[:, q4 * 512:(q4 + 1) * 512], ps3[:])
                        dst = q if pod == 0 else q2
                        sgv = sg[:].rearrange("o (hl w) -> o hl w", hl=8)
                        for hl in range(8):
                            row = chk * 8 + hl
                            drow = (dst[row:row + 1, hph, :, :] if pod == 0
                                    else dst[row:row + 1, :, :])
                            nc.gpsimd.dma_start(drow, sgv[:, hl, :])
                for ch2 in range(8):
                    osl = slice(ch2 * 8, (ch2 + 1) * 8)
                    qs = q[:, hph, osl, :]
                    q2s = q2[:, osl, :]
                    t = tts[ch2 % 2]
                    m = mms[ch2 % 2]
                    vb0 = v_sb[:, 0, hph, :].unsqueeze(1).broadcast_to([128, 8, 256])
                    tb0 = tau_sb[:, 0, hph, :].unsqueeze(1).broadcast_to([128, 8, 256])
                    vb1 = v_sb[:, 1, hph, :].unsqueeze(1).broadcast_to([128, 8, 256])
                    tb1 = tau_sb[:, 1, hph, :].unsqueeze(1).broadcast_to([128, 8, 256])
                    # pod0, in place: q <- t + min(-min(t,tau), tau) = t - clip(t)
                    nc.vector.tensor_tensor(t[:], qs, vb0, alu.mult)
                    nc.vector.tensor_tensor(m[:], t[:], tb0, alu.min)
                    nc.vector.scalar_tensor_tensor(m[:], m[:], -1.0, tb0, alu.mult, alu.min)
                    nc.vector.tensor_tensor(qs, t[:], m[:], alu.add)
                    # pod1, accumulate into q
                    nc.vector.tensor_tensor(t[:], q2s, vb1, alu.mult)
                    nc.vector.tensor_tensor(m[:], t[:], tb1, alu.min)
                    nc.vector.scalar_tensor_tensor(m[:], m[:], -1.0, tb1, alu.mult, alu.min)
                    nc.vector.tensor_tensor(t[:], t[:], m[:], alu.add)
                    nc.vector.tensor_tensor(qs, qs, t[:], alu.add)

            # ---- inverse transform, per plane ----
            for o in range(C):
                t2 = t2s[o % 2]
                for wph in range(2):
                    ps = pps.tile([128, 256], dt.float32, tag="ps")
                    nc.tensor.matmul(ps[:], q[:, 0, o, wph * 128:(wph + 1) * 128],
                                     hm_sb[:, 0, :], start=True, stop=False)
                    nc.tensor.matmul(ps[:], q[:, 1, o, wph * 128:(wph + 1) * 128],
                                     hm_sb[:, 1, :], start=False, stop=True)
                    nc.any.tensor_copy(t2[:, wph, :], ps[:])
                for hh in range(2):
                    ps = pps.tile([128, 256], dt.float32, tag="ps")
                    nc.tensor.matmul(ps[:], t2[:, 0, hh * 128:(hh + 1) * 128],
                                     hm_sb[:, 0, :], start=True, stop=False)
                    nc.tensor.matmul(ps[:], t2[:, 1, hh * 128:(hh + 1) * 128],
                                     hm_sb[:, 1, :], start=False, stop=True)
                    outp = outps[(2 * o + hh) % 4]
                    nc.any.tensor_scalar_mul(outp[:], ps[:], 16.0)
                    nc.gpsimd.dma_start(corr[b, o, hh], outp[:])

    nc.compile()
    nc.finalize()
    return nc


def _prep_consts(v, conv_w, tau):
    hm = _haar_matrix(H)
    hmt = np.ascontiguousarray(hm.T)
    hmt_s = np.ascontiguousarray(hmt.reshape(2, 128, 256).transpose(1, 0, 2)).astype(BF16)
    hm_s = np.ascontiguousarray(hm.reshape(2, 128, 256).transpose(1, 0, 2)).astype(BF16)
    wdup = np.ascontiguousarray(conv_w.transpose(2, 0, 1)).astype(BF16)  # [c, pod, o]
    v_s = np.ascontiguousarray(
        v.reshape(P, 2, 128, 256).transpose(2, 0, 1, 3)).astype(BF16)
    tau_s = np.ascontiguousarray(
        tau.reshape(P, 2, 128, 256).transpose(2, 0, 1, 3)).astype(BF16)
    return {"hmt_s": hmt_s, "hm_s": hm_s, "wdup": wdup, "v_s": v_s, "tau_s": tau_s}


def _prep_x(x):
    # [B, C, H, W] fp32 -> [B, C, 2, 128, 256] bf16 (pure reshape + cast)
    return x.reshape(B, C, 2, 128, 256).astype(BF16)


def _build_fast(nc):
    """Cached jit(shard_map) executing the bass NEFF on 8 cores.

    Same mechanism as run_bass_kernel_spmd's axon path (bass2jax custom
    call), but built once (stable jit cache) and with the NEFF output
    buffers created on-device via jnp.zeros instead of shipping
    134MB of zeros over the axon wire on every call.
    """
    import jax
    import jax.numpy as jnp
    from jax.sharding import Mesh, PartitionSpec
    try:
        from jax.experimental.shard_map import shard_map
    except ImportError:
        from jax.shard_map import shard_map  # newer jax
    from concourse import bass2jax, mybir

    bass2jax.install_neuronx_cc_hook()

    pname = nc.partition_id_tensor.name if nc.partition_id_tensor else None
    in_names, out_names, out_avals = [], [], []
    for alloc in nc.m.functions[0].allocations:
        if not isinstance(alloc, mybir.MemoryLocationSet):
            continue
        name = alloc.memorylocations[0].name
        if alloc.kind == "ExternalInput":
            if name != pname:
                in_names.append(name)
        elif alloc.kind == "ExternalOutput":
            out_names.append(name)
            out_avals.append(jax.core.ShapedArray(
                tuple(alloc.tensor_shape), mybir.dt.np(alloc.dtype)))

    bind_names = list(in_names) + list(out_names)
    if pname is not None:
        bind_names.append(pname)

    def _body(*args):
        # args = per-core inputs followed by per-core zero output buffers
        operands = list(args)
        if pname is not None:
            operands.append(bass2jax.partition_id_tensor())
        outs = bass2jax._bass_exec_p.bind(
            *operands,
            out_avals=tuple(out_avals),
            in_names=tuple(bind_names),
            out_names=tuple(out_names),
            lowering_input_output_aliases=(),
            sim_require_finite=True,
            sim_require_nnan=True,
            nc=nc,
        )
        return tuple(outs)

    devices = jax.devices()[:NCORES]
    mesh = Mesh(np.asarray(devices), ("core",))
    n_args = len(in_names) + len(out_avals)
    fast = jax.jit(shard_map(
        _body, mesh=mesh,
        in_specs=(PartitionSpec("core"),) * n_args,
        out_specs=(PartitionSpec("core"),) * len(out_names),
        check_rep=False))
    # device-resident zero output buffers, created once: passing them as
    # arguments keeps the custom-call operands parameter-only (the
    # neuronx_cc_hook rejects constants) without re-shipping 134MB of
    # zeros over the axon wire on every call.
    from jax.sharding import NamedSharding
    sh = NamedSharding(mesh, PartitionSpec("core"))
    zeros_dev = [
        jax.device_put(
            np.zeros((NCORES * a.shape[0], *a.shape[1:]), a.dtype), sh)
        for a in out_avals
    ]
    for z in zeros_dev:
        z.block_until_ready()
    return fast, in_names, out_names, zeros_dev


def _concat_inputs(in_names, per_core):
    return [np.concatenate([per_core[c][n] for c in range(NCORES)], axis=0)
            for n in in_names]


def kernel(x, v, conv_w, tau):
    x = np.asarray(x, dtype=np.float32)
    consts = _prep_consts(np.asarray(v, np.float32),
                          np.asarray(conv_w, np.float32),
                          np.asarray(tau, np.float32))
    xh = _prep_x(x)  # [B, C, 2, 128, 256] bf16, pure reshape+cast of x

    with _lock:
        if "fast" not in _state:
            from concourse.bass_utils import run_bass_kernel_spmd
            nc = _build_nc()
            xsh = xh.reshape(NCORES, BPC, C, 2, 128, 256)
            per_core = [dict(consts, xs=xsh[i]) for i in range(NCORES)]
            res = run_bass_kernel_spmd(nc, per_core, list(range(NCORES)))
            corr = np.stack([res.results[i]["corr"] for i in range(NCORES)])
            _state["fast"] = _build_fast(nc)
        else:
            fast, in_names, out_names, zeros_dev = _state["fast"]
            args = []
            for n in in_names:
                if n == "xs":
                    args.append(xh)  # [8*BPC, ...] == per-core concat
                else:
                    args.append(np.concatenate([consts[n]] * NCORES, axis=0))
            outs = fast(*args, *zeros_dev)
            corr = np.asarray(outs[out_names.index("corr")])

    # corr: [B, C, 2, 128, 256] fp8(x16), (hh, p) -> h
    corr = corr.reshape(B, C, H, W).astype(np.float32)
    corr *= (1.0 / 16.0)
    corr += x
    return corr


# revision 22
# speedup vs baseline: 3.8803x; 1.5321x over previous
"""nn_HWTConv2D Trainium2 kernel.

y = x + iHaar2d( sum_p SoftThresh( conv1x1_p( Haar2d(x) * v_p ), tau_p ) )

Haar2d(x) = HM @ X @ HM^T per (b, c) plane with the orthonormal multilevel
Haar matrix HM (256x256), so the whole pipeline is matmuls + elementwise.

Distribution: data-parallel over batch, B=16 -> 2 batches per core on 8
NeuronCores.  The device computes corr = y - x in bf16; the host adds the
fp32 residual x, which keeps the wire traffic at bf16 both ways (the axon
tunnel at ~60 MB/s is the real bottleneck, not the device).

Per-core bass program (all matmuls bf16, PSUM fp32):
  S1/S2  forward transform, data-stationary matmuls (lhsT = data tile,
         rhs = HM^T tiles): each stage contracts the partition dim and
         transposes the plane, so two stages return to [h'|w'] layout.
  conv   channels must sit on partitions: small DMA gathers build
         [c=64 | pix] tiles from F1, one stationary W^T per pod, and the
         [o | pix] results are DMA-scattered back to [h' | (o, w')].
  thresh f5 = t - clip(t, +-tau) with t = q * v, computed as
         m = min(t, tau); m2 = min(-m, tau) = -clip; f5 = t + m2,
         using v/tau slices broadcast along the free (o) dim.
  I_h/I_w inverse transform, same data-stationary structure, writes
         corr planes [h | w] contiguously to HBM.
"""

import threading

import numpy as np
import ml_dtypes

B, C, H, W, P = 16, 64, 256, 256, 2
NCORES = 8
BPC = B // NCORES  # batches per core
BF16 = ml_dtypes.bfloat16
NORM = float(1.0 / np.sqrt(2.0))

_lock = threading.Lock()
_state: dict = {}


def _haar_matrix(n):
    # Orthonormal multilevel 1D Haar matrix: haar1d_fwd(x) == HM @ x.
    m = int(np.log2(n))
    hm = np.eye(n, dtype=np.float64)
    length = n
    for _ in range(m):
        lvl = np.eye(n, dtype=np.float64)
        half = length // 2
        blk = np.zeros((length, length), dtype=np.float64)
        for i in range(half):
            blk[i, 2 * i] = NORM
            blk[i, 2 * i + 1] = NORM
            blk[half + i, 2 * i] = NORM
            blk[half + i, 2 * i + 1] = -NORM
        lvl[:length, :length] = blk
        hm = lvl @ hm
        length //= 2
    return hm.astype(np.float32)


def _build_nc():
    import concourse.bacc as bacc
    import concourse.tile as tile
    from concourse import mybir
    from contextlib import ExitStack

    dt = mybir.dt
    alu = mybir.AluOpType
    nc = bacc.Bacc("TRN2", target_bir_lowering=False, debug=False)

    xs = nc.dram_tensor("xs", [BPC, C, 2, 128, 256], dt.float8e4, kind="ExternalInput")
    hmt_s = nc.dram_tensor("hmt_s", [128, 2, 256], dt.bfloat16, kind="ExternalInput")
    hm_s = nc.dram_tensor("hm_s", [128, 2, 256], dt.bfloat16, kind="ExternalInput")
    wdup = nc.dram_tensor("wdup", [64, P, 64], dt.bfloat16, kind="ExternalInput")
    v_s = nc.dram_tensor("v_s", [128, P, 2, 256], dt.bfloat16, kind="ExternalInput")
    tau_s = nc.dram_tensor("tau_s", [128, P, 2, 256], dt.bfloat16, kind="ExternalInput")
    corr = nc.dram_tensor("corr", [BPC, C, 2, 128, 256], dt.float8e4, kind="ExternalOutput")

    with ExitStack() as ctx:
        tc = ctx.enter_context(tile.TileContext(nc))
        pc = ctx.enter_context(tc.tile_pool(name="consts", bufs=1))
        pw = ctx.enter_context(tc.tile_pool(name="work", bufs=1))
        pps = ctx.enter_context(tc.tile_pool(name="ps", bufs=4, space="PSUM"))
        pps3 = ctx.enter_context(tc.tile_pool(name="ps3", bufs=4, space="PSUM"))

        hmt_sb = pc.tile_from(hmt_s[:])
        hm_sb = pc.tile_from(hm_s[:])
        wdup_sb = pc.tile_from(wdup[:])
        v_sb = pc.tile_from(v_s[:])
        tau_sb = pc.tile_from(tau_s[:])

        # Every working tile is allocated exactly once (static address) and
        # reused round-robin: only same-tensor dependency tracking is needed.
        f1 = pw.tile([128, 2, C, 256], dt.bfloat16, tag="f1")
        q = pw.tile([128, 2, C, 256], dt.bfloat16, tag="q")
        q2 = pw.tile([128, C, 256], dt.bfloat16, tag="q2")
        xcs = [pw.tile([128, 2, 256], dt.float8e4, tag=f"xc{i}", name=f"xc{i}") for i in range(3)]
        r1s_ = [pw.tile([128, 256], dt.bfloat16, tag=f"r1{i}", name=f"r1{i}") for i in range(4)]
        ftcs = [pw.tile([64, 8 * 256], dt.bfloat16, tag=f"ftc{i}", name=f"ftc{i}") for i in range(2)]
        sgs = [pw.tile([64, 8 * 256], dt.bfloat16, tag=f"sg{i}", name=f"sg{i}") for i in range(2)]
        tts = [pw.tile([128, 8, 256], dt.bfloat16, tag=f"tt{i}", name=f"tt{i}") for i in range(2)]
        mms = [pw.tile([128, 8, 256], dt.bfloat16, tag=f"mm{i}", name=f"mm{i}") for i in range(2)]
        t2s = [pw.tile([128, 2, 256], dt.bfloat16, tag=f"t2{i}", name=f"t2{i}") for i in range(2)]
        outps = [pw.tile([128, 256], dt.float8e4, tag=f"outp{i}", name=f"outp{i}") for i in range(4)]

        for b in range(BPC):
            # ---- forward transform: per-plane, fused S1+S2 ----
            for c in range(C):
                xc = xcs[c % 3]
                for hh in range(2):
                    nc.gpsimd.dma_start(xc[:, hh, :], xs[b, c, hh])
                r1s = []
                for wh in range(2):
                    ps1 = pps.tile([128, 256], dt.float32, tag="ps")
                    nc.tensor.matmul(ps1[:], xc[:, 0, wh * 128:(wh + 1) * 128],
                                     hmt_sb[:, 0, :], start=True, stop=False)
                    nc.tensor.matmul(ps1[:], xc[:, 1, wh * 128:(wh + 1) * 128],
                                     hmt_sb[:, 1, :], start=False, stop=True)
                    r1 = r1s_[(2 * c + wh) % 4]
                    nc.any.tensor_copy(r1[:], ps1[:])
                    r1s.append(r1)
                for hph in range(2):
                    ps2 = pps.tile([128, 256], dt.float32, tag="ps")
                    nc.tensor.matmul(ps2[:], r1s[0][:, hph * 128:(hph + 1) * 128],
                                     hmt_sb[:, 0, :], start=True, stop=False)
                    nc.tensor.matmul(ps2[:], r1s[1][:, hph * 128:(hph + 1) * 128],
                                     hmt_sb[:, 1, :], start=False, stop=True)
                    nc.any.tensor_copy(f1[:, hph, c, :], ps2[:])

            # ---- conv (channel mix) + soft-threshold ----
            for hph in range(2):
                for chk in range(16):
                    ftc = ftcs[chk % 2]
                    ftv = ftc[:].rearrange("c (hl w) -> c hl w", hl=8)
                    for hl in range(8):
                        row = chk * 8 + hl
                        nc.gpsimd.dma_start(ftv[:, hl, :], f1[row:row + 1, hph, :, :])
                    for pod in range(P):
                        sg = sgs[pod]
                        for q4 in range(4):
                            ps3 = pps3.tile([64, 512], dt.float32, tag="ps3")
                            nc.tensor.matmul(ps3[:], wdup_sb[:, pod, :],
                                             ftc[:, q4 * 512:(q4 + 1) * 512],
                                             start=True, stop=True)
                            nc.any.tensor_copy(sg[:, q4 * 512:(q4 + 1) * 512], ps3[:])
                        dst = q if pod == 0 else q2
                        sgv = sg[:].rearrange("o (hl w) -> o hl w", hl=8)
                        for hl in range(8):
                            row = chk * 8 + hl
                            drow = (dst[row:row + 1, hph, :, :] if pod == 0
                                    else dst[row:row + 1, :, :])
                            nc.gpsimd.dma_start(drow, sgv[:, hl, :])
                for ch2 in range(8):
                    osl = slice(ch2 * 8, (ch2 + 1) * 8)
                    qs = q[:, hph, osl, :]
                    q2s = q2[:, osl, :]
                    t = tts[ch2 % 2]
                    m = mms[ch2 % 2]
                    vb0 = v_sb[:, 0, hph, :].unsqueeze(1).broadcast_to([128, 8, 256])
                    tb0 = tau_sb[:, 0, hph, :].unsqueeze(1).broadcast_to([128, 8, 256])
                    vb1 = v_sb[:, 1, hph, :].unsqueeze(1).broadcast_to([128, 8, 256])
                    tb1 = tau_sb[:, 1, hph, :].unsqueeze(1).broadcast_to([128, 8, 256])
                    # pod0, in place: q <- t + min(-min(t,tau), tau) = t - clip(t)
                    nc.vector.tensor_tensor(t[:], qs, vb0, alu.mult)
                    nc.vector.tensor_tensor(m[:], t[:], tb0, alu.min)
                    nc.vector.scalar_tensor_tensor(m[:], m[:], -1.0, tb0, alu.mult, alu.min)
                    nc.vector.tensor_tensor(qs, t[:], m[:], alu.add)
                    # pod1, accumulate into q
                    nc.vector.tensor_tensor(t[:], q2s, vb1, alu.mult)
                    nc.vector.tensor_tensor(m[:], t[:], tb1, alu.min)
                    nc.vector.scalar_tensor_tensor(m[:], m[:], -1.0, tb1, alu.mult, alu.min)
                    nc.vector.tensor_tensor(t[:], t[:], m[:], alu.add)
                    nc.vector.tensor_tensor(qs, qs, t[:], alu.add)

            # ---- inverse transform, per plane ----
            for o in range(C):
                t2 = t2s[o % 2]
                for wph in range(2):
                    ps = pps.tile([128, 256], dt.float32, tag="ps")
                    nc.tensor.matmul(ps[:], q[:, 0, o, wph * 128:(wph + 1) * 128],
                                     hm_sb[:, 0, :], start=True, stop=False)
                    nc.tensor.matmul(ps[:], q[:, 1, o, wph * 128:(wph + 1) * 128],
                                     hm_sb[:, 1, :], start=False, stop=True)
                    nc.any.tensor_copy(t2[:, wph, :], ps[:])
                for hh in range(2):
                    ps = pps.tile([128, 256], dt.float32, tag="ps")
                    nc.tensor.matmul(ps[:], t2[:, 0, hh * 128:(hh + 1) * 128],
                                     hm_sb[:, 0, :], start=True, stop=False)
                    nc.tensor.matmul(ps[:], t2[:, 1, hh * 128:(hh + 1) * 128],
                                     hm_sb[:, 1, :], start=False, stop=True)
                    outp = outps[(2 * o + hh) % 4]
                    nc.any.tensor_scalar_mul(outp[:], ps[:], 16.0)
                    nc.gpsimd.dma_start(corr[b, o, hh], outp[:])

    nc.compile()
    nc.finalize()
    return nc


def _prep_consts(v, conv_w, tau):
    hm = _haar_matrix(H)
    hmt = np.ascontiguousarray(hm.T)
    hmt_s = np.ascontiguousarray(hmt.reshape(2, 128, 256).transpose(1, 0, 2)).astype(BF16)
    hm_s = np.ascontiguousarray(hm.reshape(2, 128, 256).transpose(1, 0, 2)).astype(BF16)
    wdup = np.ascontiguousarray(conv_w.transpose(2, 0, 1)).astype(BF16)  # [c, pod, o]
    v_s = np.ascontiguousarray(
        v.reshape(P, 2, 128, 256).transpose(2, 0, 1, 3)).astype(BF16)
    tau_s = np.ascontiguousarray(
        tau.reshape(P, 2, 128, 256).transpose(2, 0, 1, 3)).astype(BF16)
    return {"hmt_s": hmt_s, "hm_s": hm_s, "wdup": wdup, "v_s": v_s, "tau_s": tau_s}


FP8 = ml_dtypes.float8_e4m3


def _prep_x(x):
    # [B, C, H, W] fp32 -> [B, C, 2, 128, 256] fp8 (pure reshape + cast)
    buf = _state.get("xh_buf")
    if buf is None:
        buf = _state["xh_buf"] = np.empty((B, C, 2, 128, 256), dtype=FP8)
    buf[...] = x.reshape(B, C, 2, 128, 256)
    return buf


def _build_fast(nc):
    """Cached jit(shard_map) executing the bass NEFF on 8 cores.

    Same mechanism as run_bass_kernel_spmd's axon path (bass2jax custom
    call), but built once (stable jit cache) and with the NEFF output
    buffers created on-device via jnp.zeros instead of shipping
    134MB of zeros over the axon wire on every call.
    """
    import jax
    import jax.numpy as jnp
    from jax.sharding import Mesh, PartitionSpec
    try:
        from jax.experimental.shard_map import shard_map
    except ImportError:
        from jax.shard_map import shard_map  # newer jax
    from concourse import bass2jax, mybir

    bass2jax.install_neuronx_cc_hook()

    pname = nc.partition_id_tensor.name if nc.partition_id_tensor else None
    in_names, out_names, out_avals = [], [], []
    for alloc in nc.m.functions[0].allocations:
        if not isinstance(alloc, mybir.MemoryLocationSet):
            continue
        name = alloc.memorylocations[0].name
        if alloc.kind == "ExternalInput":
            if name != pname:
                in_names.append(name)
        elif alloc.kind == "ExternalOutput":
            out_names.append(name)
            out_avals.append(jax.core.ShapedArray(
                tuple(alloc.tensor_shape), mybir.dt.np(alloc.dtype)))

    bind_names = list(in_names) + list(out_names)
    if pname is not None:
        bind_names.append(pname)

    def _body(*args):
        # args = per-core inputs followed by per-core zero output buffers
        operands = list(args)
        if pname is not None:
            operands.append(bass2jax.partition_id_tensor())
        outs = bass2jax._bass_exec_p.bind(
            *operands,
            out_avals=tuple(out_avals),
            in_names=tuple(bind_names),
            out_names=tuple(out_names),
            lowering_input_output_aliases=(),
            sim_require_finite=True,
            sim_require_nnan=True,
            nc=nc,
        )
        return tuple(outs)

    devices = jax.devices()[:NCORES]
    mesh = Mesh(np.asarray(devices), ("core",))
    n_args = len(in_names) + len(out_avals)
    fast = jax.jit(shard_map(
        _body, mesh=mesh,
        in_specs=(PartitionSpec("core"),) * n_args,
        out_specs=(PartitionSpec("core"),) * len(out_names),
        check_rep=False))
    # device-resident zero output buffers, created once: passing them as
    # arguments keeps the custom-call operands parameter-only (the
    # neuronx_cc_hook rejects constants) without re-shipping 134MB of
    # zeros over the axon wire on every call.
    from jax.sharding import NamedSharding
    sh = NamedSharding(mesh, PartitionSpec("core"))
    zeros_dev = [
        jax.device_put(
            np.zeros((NCORES * a.shape[0], *a.shape[1:]), a.dtype), sh)
        for a in out_avals
    ]
    for z in zeros_dev:
        z.block_until_ready()
    return fast, in_names, out_names, zeros_dev


def _concat_inputs(in_names, per_core):
    return [np.concatenate([per_core[c][n] for c in range(NCORES)], axis=0)
            for n in in_names]


def kernel(x, v, conv_w, tau):
    x = np.asarray(x, dtype=np.float32)
    consts = _prep_consts(np.asarray(v, np.float32),
                          np.asarray(conv_w, np.float32),
                          np.asarray(tau, np.float32))
    xh = _prep_x(x)  # [B, C, 2, 128, 256] bf16, pure reshape+cast of x

    with _lock:
        if "fast" not in _state:
            from concourse.bass_utils import run_bass_kernel_spmd
            nc = _build_nc()
            xsh = xh.reshape(NCORES, BPC, C, 2, 128, 256)
            per_core = [dict(consts, xs=xsh[i]) for i in range(NCORES)]
            res = run_bass_kernel_spmd(nc, per_core, list(range(NCORES)))
            corr = np.stack([res.results[i]["corr"] for i in range(NCORES)])
            _state["fast"] = _build_fast(nc)
        else:
            fast, in_names, out_names, zeros_dev = _state["fast"]
            args = []
            for n in in_names:
                if n == "xs":
                    args.append(xh)  # [8*BPC, ...] == per-core concat
                else:
                    args.append(np.concatenate([consts[n]] * NCORES, axis=0))
            outs = fast(*args, *zeros_dev)
            corr = np.asarray(outs[out_names.index("corr")])

    # corr: [B, C, 2, 128, 256] fp8(x16), (hh, p) -> h
    ybuf = _state.get("y_buf")
    if ybuf is None:
        ybuf = _state["y_buf"] = np.empty((B, C, H, W), dtype=np.float32)
    ybuf[...] = corr.reshape(B, C, H, W)
    ybuf *= (1.0 / 16.0)
    ybuf += x
    return ybuf
